# revision 1
# baseline (speedup 1.0000x reference)
"""Self-contained Trainium2 kernel for nn_LoadBalancingLoss.

Strategy: data-parallel over 8 NeuronCores (batch shard). Each core computes
a 64-bin histogram (per-expert counts and gate sums) of its 2.1M (gate, idx)
pairs via masked fused-reduce passes on the Vector engine; the tiny per-core
partials are summed on host (fp64) and turned into the scalar loss.

Inputs (full): gates [64, 32768, 8] fp32, indices [64, 32768, 8] int64.
Output: scalar fp32 loss.
"""

from contextlib import ExitStack

import numpy as np

import concourse.bacc as bacc
import concourse.bass as bass
import concourse.mybir as mybir
from concourse import tile
from concourse.bass_utils import run_bass_kernel_spmd

F32 = mybir.dt.float32
I32 = mybir.dt.int32

NB = 64          # experts
N_CORES = 8
B, S, K = 64, 32768, 8
FT = (B // N_CORES) * S * K // 128   # free elems per partition per core: 16384
NCH = 4          # processing chunks per core
TOTAL = B * S * K

_CACHE: dict = {}


def _emit(tc, gates_d, idx32_d, out_d):
    nc = tc.nc
    F = FT // NCH
    with ExitStack() as ctx:
        raw_pool = ctx.enter_context(tc.tile_pool(name="raw", bufs=2))
        g_pool = ctx.enter_context(tc.tile_pool(name="g", bufs=2))
        if_pool = ctx.enter_context(tc.tile_pool(name="idxf", bufs=2))
        scr_pool = ctx.enter_context(tc.tile_pool(name="scr", bufs=1))
        one_pool = ctx.enter_context(tc.tile_pool(name="ones", bufs=1))
        acc_pool = ctx.enter_context(tc.tile_pool(name="acc", bufs=2))

        ones = one_pool.tile([128, F], F32)
        nc.vector.memset(ones[:], 1.0)

        for c in range(NCH):
            raw = raw_pool.tile([128, 2 * F], I32)
            nc.sync.dma_start(out=raw[:], in_=idx32_d[:, c * 2 * F:(c + 1) * 2 * F])
            g = g_pool.tile([128, F], F32)
            nc.sync.dma_start(out=g[:], in_=gates_d[:, c * F:(c + 1) * F])

            idxf = if_pool.tile([128, F], F32)
            raw_lo = raw[:].rearrange("p (f two) -> p f two", two=2)[:, :, 0]
            nc.vector.tensor_copy(idxf[:], raw_lo)

            scr = scr_pool.tile([128, F], F32)
            acc = acc_pool.tile([128, 2 * NB], F32)
            for e in range(NB):
                nc.vector.scalar_tensor_tensor(
                    out=scr[:], in0=idxf[:], scalar=float(e), in1=g[:],
                    op0=mybir.AluOpType.is_equal, op1=mybir.AluOpType.mult,
                    accum_out=acc[:, e:e + 1],
                )
            for e in range(NB):
                nc.vector.scalar_tensor_tensor(
                    out=scr[:], in0=idxf[:], scalar=float(e), in1=ones[:],
                    op0=mybir.AluOpType.is_equal, op1=mybir.AluOpType.mult,
                    accum_out=acc[:, NB + e:NB + e + 1],
                )
            nc.sync.dma_start(
                out=out_d[:, c * 2 * NB:(c + 1) * 2 * NB], in_=acc[:])


def _build():
    if "nc" in _CACHE:
        return _CACHE["nc"]
    nc = bacc.Bacc(trn_type="TRN2", num_devices=N_CORES)
    gates_d = nc.dram_tensor("gates", [128, FT], F32, kind="ExternalInput")
    idx32_d = nc.dram_tensor("idx32", [128, 2 * FT], I32, kind="ExternalInput")
    out_d = nc.dram_tensor("out", [128, 2 * NB * NCH], F32, kind="ExternalOutput")
    with tile.TileContext(nc) as tc:
        _emit(tc, gates_d.ap(), idx32_d.ap(), out_d.ap())
    nc.compile()
    _CACHE["nc"] = nc
    return nc


def _shard(gates: np.ndarray, indices: np.ndarray):
    bc = B // N_CORES
    in_maps = []
    for i in range(N_CORES):
        g = np.ascontiguousarray(gates[i * bc:(i + 1) * bc], dtype=np.float32)
        ix = np.ascontiguousarray(indices[i * bc:(i + 1) * bc])
        if ix.dtype != np.int64:
            ix = ix.astype(np.int64)
        n = g.size
        in_maps.append({
            "gates": g.reshape(128, n // 128),
            "idx32": ix.view(np.int32).reshape(128, 2 * (n // 128)),
        })
    return in_maps


def _combine(outs) -> np.float32:
    s = np.zeros(NB, np.float64)
    cnt = np.zeros(NB, np.float64)
    for o in outs:
        o = o.astype(np.float64).reshape(128, NCH, 2 * NB)
        s += o[:, :, :NB].sum(axis=(0, 1))
        cnt += o[:, :, NB:].sum(axis=(0, 1))
    return np.float32(NB * np.sum((cnt / TOTAL) * (s / TOTAL)))


def kernel(gates: np.ndarray, indices: np.ndarray) -> np.float32:
    gates = np.asarray(gates)
    indices = np.asarray(indices)
    nc = _build()
    in_maps = _shard(gates, indices)
    res = run_bass_kernel_spmd(nc, in_maps, core_ids=list(range(N_CORES)))
    return _combine([res.results[i]["out"] for i in range(N_CORES)])


# ---------------------------------------------------------------- timing ----

def time_kernel(gates: np.ndarray, indices: np.ndarray, iters: int = 20) -> float:
    """Best per-call wall time (ns) of the compiled 8-core executable with
    device-resident inputs. Upper bound on HW exec time."""
    import time

    import jax
    from jax.sharding import Mesh, NamedSharding, PartitionSpec

    from concourse import bass2jax

    nc = _build()
    in_maps = _shard(np.asarray(gates), np.asarray(indices))
    bass2jax.install_neuronx_cc_hook()

    partition_name = nc.partition_id_tensor.name if nc.partition_id_tensor else None
    in_names, out_names, out_avals, zero_outs = [], [], [], []
    for alloc in nc.m.functions[0].allocations:
        if not isinstance(alloc, mybir.MemoryLocationSet):
            continue
        name = alloc.memorylocations[0].name
        if alloc.kind == "ExternalInput":
            if name != partition_name:
                in_names.append(name)
        elif alloc.kind == "ExternalOutput":
            shape = tuple(alloc.tensor_shape)
            dtype = mybir.dt.np(alloc.dtype)
            out_names.append(name)
            out_avals.append(jax.core.ShapedArray(shape, dtype))
            zero_outs.append(np.zeros(shape, dtype))
    n_params = len(in_names)
    n_outs = len(out_avals)
    all_in_names = list(in_names) + list(out_names)
    if partition_name is not None:
        all_in_names.append(partition_name)
    donate = tuple(range(n_params, n_params + n_outs))

    def _body(*args):
        operands = list(args)
        if partition_name is not None:
            operands.append(bass2jax.partition_id_tensor())
        outs = bass2jax._bass_exec_p.bind(
            *operands,
            out_avals=tuple(out_avals),
            in_names=tuple(all_in_names),
            out_names=tuple(out_names),
            lowering_input_output_aliases=(),
            sim_require_finite=True,
            sim_require_nnan=True,
            nc=nc,
        )
        return tuple(outs)

    devices = jax.devices()[:N_CORES]
    mesh = Mesh(np.asarray(devices), ("core",))
    from jax.experimental.shard_map import shard_map
    in_specs = (PartitionSpec("core"),) * (n_params + n_outs)
    out_specs = (PartitionSpec("core"),) * n_outs
    sharded = jax.jit(
        shard_map(_body, mesh=mesh, in_specs=in_specs, out_specs=out_specs,
                  check_rep=False),
        donate_argnums=donate, keep_unused=True)

    sh = NamedSharding(mesh, PartitionSpec("core"))
    concat_in = [
        jax.device_put(
            np.concatenate([in_maps[c][nm] for c in range(N_CORES)], axis=0), sh)
        for nm in in_names
    ]

    def fresh_zeros():
        return [jax.device_put(
            np.zeros((N_CORES * z.shape[0], *z.shape[1:]), z.dtype), sh)
            for z in zero_outs]

    # warmup (compile)
    outs = sharded(*concat_in, *fresh_zeros())
    jax.block_until_ready(outs)

    best = float("inf")
    for _ in range(iters):
        zs = fresh_zeros()
        jax.block_until_ready(zs)
        t0 = time.perf_counter()
        outs = sharded(*concat_in, *zs)
        jax.block_until_ready(outs)
        t1 = time.perf_counter()
        best = min(best, t1 - t0)
    return best * 1e9
